# revision 8
# baseline (speedup 1.0000x reference)
"""Trainium2 Bass kernel for Mixtral-style MoE (8 experts, top-2, SwiGLU).

Strategy: expert-parallel across the 8 NeuronCores with host-side dispatch.
The router is tiny (8192x2048x8 = 0.27 GFLOP) and runs on host CPU with the
exact same jax ops as the reference (bitwise-matching top-2 selection).  Each
core owns one expert: the host gathers that expert's routed tokens (avg 2048
of the 16384 (token, expert) pairs), pads to a uniform capacity C=2048 so all
cores run the same program (SPMD), and the device does only the expert FFN:

  h1 = W1 @ x ; h3 = W3 @ x ; h = silu(h1) * h3 ; y = (W2 @ h) * pair_weight

in bf16 with fp32 PSUM accumulation (103 GFLOP/core vs 412 dense).  The host
scatter-adds each token's two expert partials.

Mixed precision: slots are filled per expert in DESCENDING pair-weight order.
The last 512 slots (the expert's lowest-weight pairs, mostly the rank-2
expert of well-routed tokens) run GEMM2 in fp8-e4m3 DoubleRow mode (2x PE
throughput): h is cast to e4m3 unscaled (|h| << 240), w2 is pre-scaled by
2048 on host (absmax*2048 = 222 < 240) and the 1/2048 unscale is folded into
the host-prepared pair-weight matrix.  Measured end-to-end rel err 1.2e-2
against the fp32 reference (gate 2e-2); bf16-only is 4.1e-3.

Token chunks are processed in PAIRS per weight pass: each streamed weight
tile issues two back-to-back matmuls (chunk A, chunk B) so the stationary-
operand load amortizes over 1024 moving columns, and weight HBM traffic
halves.  Other schedule details:
  - w1/w3 slabs are packed into ONE [it, 128, 2H] stream tile: one DMA + one
    PE semaphore wait per i-tile.
  - the stream is prefetched 2 tiles deep BEFORE the x-tile DMA block.
  - GEMM2's PSUM pool is double-buffered (no stall at hh boundaries).
  - the pair-weight broadcast [128, C] is precomputed on host and DMA'd
    outside the warmup critical path.

Layouts (host-prepared, per core e):
  xg    : [H, C]   bf16 gathered tokens, slot-ordered (weight-desc)
  wb    : [128, C] fp32 pair weight broadcast along partitions (0 on padding;
          divided by 2048 on fp8 slots)
  w13t  : [I/128, 128, 2H] bf16; first H cols w1 lhsT slab, last H w3
          (slab i row p, col k*128+c holds w[i*128+c, k*128+p])
  w2t   : [H/128, 128, I] bf16, same blocking for w2.T
  w2t8  : [H/128, 128, I/256, 2, 128] e4m3, DoubleRow blocking of w2.T*2048:
          [hh, p, j, r, m] = w2[hh*128+m, j*256+r*128+p]*2048
  out   : [H, C] fp32 partial outputs (host transposes/scatter-adds)
"""

import numpy as np
import ml_dtypes

import concourse.bass as bass
import concourse.mybir as mybir
import concourse.tile as tile
from concourse import bacc

P = 128
FP32 = mybir.dt.float32
BF16 = mybir.dt.bfloat16
E4 = mybir.dt.float8e4

# Full-problem constants
N_CORES = 8
NUM_TOKENS = 8192
HIDDEN = 2048
INTER = 4096
EXPERTS = 8
TOP_K = 2

W2_FP8_SCALE = 2048.0   # power of two; w2 absmax * 2048 = 222 < 240


def build_program(groups, lo_chunks=frozenset(), h=HIDDEN, i_sz=INTER):
    """groups: tuple of tuples of chunk sizes.  Each group is either
    (a,) / (a, b) with a,b <= 512, or (a, b, t) with t <= 256 (tail rider).
    lo_chunks: set of (group_idx, chunk_idx) whose GEMM2 runs in fp8
    DoubleRow (chunk size must be 512 and it must be even for those).
    """
    c_cap = sum(sum(g) for g in groups)
    kt = h // P
    it = i_sz // P
    ht = h // P

    nc = bacc.Bacc("TRN2", target_bir_lowering=False, debug=False)

    xg = nc.dram_tensor("xg", [h, c_cap], BF16, kind="ExternalInput").ap()
    wb_d = nc.dram_tensor("wb", [P, c_cap], FP32, kind="ExternalInput").ap()
    w13t = nc.dram_tensor("w13t", [it, P, 2 * h], BF16, kind="ExternalInput").ap()
    w2t = nc.dram_tensor("w2t", [ht, P, i_sz], BF16, kind="ExternalInput").ap()
    need_fp8 = bool(lo_chunks)
    if need_fp8:
        w2t8 = nc.dram_tensor("w2t8", [ht, P, it // 2, 2, P], E4,
                              kind="ExternalInput").ap()
    out_d = nc.dram_tensor("out", [h, c_cap], FP32, kind="ExternalOutput").ap()

    with tile.TileContext(nc) as tc:
        with (
            tc.tile_pool(name="const", bufs=1) as const_pool,
            tc.tile_pool(name="xpool", bufs=1) as x_pool,
            tc.tile_pool(name="hpool", bufs=1) as h_pool,
            tc.tile_pool(name="stream", bufs=3) as stream_pool,
            tc.tile_pool(name="w2stream", bufs=2) as w2_pool,
            tc.tile_pool(name="work", bufs=2) as work_pool,
            tc.tile_pool(name="opool", bufs=3) as o_pool,
            tc.tile_pool(name="psum1", bufs=1, space="PSUM") as psum1,
            tc.tile_pool(name="psum2", bufs=2, space="PSUM") as psum2,
        ):
            first_grp = True
            wb_full = None

            off = 0
            for gi, grp in enumerate(groups):
                cks = []
                for ck in grp:
                    cks.append((off, ck))
                    off += ck
                nch = len(cks)
                has_tail = nch == 3
                if has_tail:
                    assert cks[2][1] <= 256
                is_lo = [(gi, c) in lo_chunks for c in range(nch)]
                for c in range(nch):
                    if is_lo[c]:
                        assert cks[c][1] == 512 and it % 2 == 0

                # 2-deep prefetch of the merged w1/w3 stream, issued before
                # the x block so the first matmul's stationary tile is in
                # flight immediately.
                w13_pend = []

                def issue_w13(i):
                    t = stream_pool.tile([P, 2 * h], BF16, tag="w13s",
                                         name="w13s")
                    nc.sync.dma_start(out=t[:], in_=w13t[i])
                    w13_pend.append(t)

                issue_w13(0)
                issue_w13(1)

                # x tiles, k-major to match matmul consumption order
                xtb = [[None] * kt for _ in range(nch)]
                for k in range(kt):
                    for c, (o, ck) in enumerate(cks):
                        x = x_pool.tile([P, ck], BF16, tag=f"xtb{c}_{k}",
                                        name=f"xtb{c}_{k}")
                        nc.scalar.dma_start(
                            out=x[:], in_=xg[k * P:(k + 1) * P, o:o + ck])
                        xtb[c][k] = x

                # GEMM1 + SwiGLU
                h_sb = [[] for _ in range(nch)]     # bf16 h tiles (hi chunks)
                h8_sb = [[] for _ in range(nch)]    # e4m3 DoubleRow tiles
                for i in range(it):
                    if i + 2 < it:
                        issue_w13(i + 2)
                    w13s = w13_pend.pop(0)
                    h1_ps, h3_ps = [], []
                    for c, (_, ck) in enumerate(cks):
                        if c == 2:  # tail rider: one bank, two halves
                            hT = psum1.tile([P, 2 * ck], FP32, tag="hT",
                                            name="hT")
                            h1_ps.append(hT[:, 0:ck])
                            h3_ps.append(hT[:, ck:2 * ck])
                        else:
                            t1 = psum1.tile([P, ck], FP32, tag=f"h1_{c}",
                                            name=f"h1_{c}")
                            t3 = psum1.tile([P, ck], FP32, tag=f"h3_{c}",
                                            name=f"h3_{c}")
                            h1_ps.append(t1[:])
                            h3_ps.append(t3[:])
                    for k in range(kt):
                        for c in range(nch):
                            nc.tensor.matmul(out=h1_ps[c],
                                             lhsT=w13s[:, k * P:(k + 1) * P],
                                             rhs=xtb[c][k][:],
                                             start=(k == 0), stop=(k == kt - 1))
                    for k in range(kt):
                        for c in range(nch):
                            nc.tensor.matmul(
                                out=h3_ps[c],
                                lhsT=w13s[:, h + k * P:h + (k + 1) * P],
                                rhs=xtb[c][k][:],
                                start=(k == 0), stop=(k == kt - 1))
                    for c, (_, ck) in enumerate(cks):
                        sg = work_pool.tile([P, ck], FP32, tag=f"sg{c}",
                                            name=f"sg{c}")
                        nc.scalar.activation(
                            out=sg[:], in_=h1_ps[c],
                            func=mybir.ActivationFunctionType.Sigmoid)
                        sil = work_pool.tile([P, ck], FP32, tag=f"sil{c}",
                                             name=f"sil{c}")
                        nc.vector.tensor_tensor(out=sil[:], in0=sg[:],
                                                in1=h1_ps[c],
                                                op=mybir.AluOpType.mult)
                        if is_lo[c]:
                            # DoubleRow rhs tile: pair j holds i=2j (r=0) and
                            # i=2j+1 (r=1); h cast straight to e4m3 (no scale)
                            j, r = divmod(i, 2)
                            if r == 0:
                                h8 = h_pool.tile([P, 2, ck], E4,
                                                 tag=f"h8_{c}_{j}",
                                                 name=f"h8_{c}_{j}")
                                h8_sb[c].append(h8)
                            h8 = h8_sb[c][j]
                            nc.vector.tensor_tensor(out=h8[:, r, :], in0=sil[:],
                                                    in1=h3_ps[c],
                                                    op=mybir.AluOpType.mult)
                        else:
                            hcur = h_pool.tile([P, ck], BF16, tag=f"h{c}_{i}",
                                               name=f"h{c}_{i}")
                            nc.vector.tensor_tensor(out=hcur[:], in0=sil[:],
                                                    in1=h3_ps[c],
                                                    op=mybir.AluOpType.mult)
                            h_sb[c].append(hcur)

                if first_grp:
                    # pair-weight broadcast, precomputed on host; DMA'd here
                    # (first needed at the first yo multiply) so it stays off
                    # the scalar DMA ring's critical lead path.
                    wb_full = const_pool.tile([P, c_cap], FP32, tag="wb_full")
                    nc.scalar.dma_start(out=wb_full[:], in_=wb_d[:, :])
                    first_grp = False

                # GEMM2, w2 streams prefetched 2 deep
                any_hi = any(not lo for lo in is_lo)
                any_lo = any(is_lo)
                w2_pend = []
                w28_pend = []

                def issue_w2(hh):
                    if any_hi:
                        t = w2_pool.tile([P, i_sz], BF16, tag="w2s",
                                         name="w2s")
                        nc.sync.dma_start(out=t[:], in_=w2t[hh])
                        w2_pend.append(t)
                    if any_lo:
                        t8 = w2_pool.tile([P, it // 2, 2, P], E4, tag="w28s",
                                          name="w28s")
                        nc.sync.dma_start(out=t8[:], in_=w2t8[hh])
                        w28_pend.append(t8)

                issue_w2(0)
                issue_w2(1)
                for hh in range(ht):
                    if hh + 2 < ht:
                        issue_w2(hh + 2)
                    w2s = w2_pend.pop(0) if any_hi else None
                    w28s = w28_pend.pop(0) if any_lo else None
                    f_ps = [psum2.tile([P, ck], FP32, tag=f"f_{c}",
                                       name=f"f_{c}")
                            for c, (_, ck) in enumerate(cks)]
                    for c in range(nch):
                        if is_lo[c]:
                            continue
                        for i in range(it):
                            nc.tensor.matmul(out=f_ps[c][:],
                                             lhsT=w2s[:, i * P:(i + 1) * P],
                                             rhs=h_sb[c][i][:],
                                             start=(i == 0), stop=(i == it - 1))
                    for c in range(nch):
                        if not is_lo[c]:
                            continue
                        for j in range(it // 2):
                            nc.tensor.matmul(
                                out=f_ps[c][:], lhsT=w28s[:, j],
                                rhs=h8_sb[c][j][:],
                                start=(j == 0), stop=(j == it // 2 - 1),
                                perf_mode=mybir.MatmulPerfMode.DoubleRow)
                    for c, (o, ck) in enumerate(cks):
                        yo = o_pool.tile([P, ck], FP32, tag=f"yo{c}",
                                         name=f"yo{c}")
                        nc.vector.tensor_tensor(out=yo[:], in0=f_ps[c][:],
                                                in1=wb_full[:, o:o + ck],
                                                op=mybir.AluOpType.mult)
                        nc.scalar.dma_start(
                            out=out_d[hh * P:(hh + 1) * P, o:o + ck],
                            in_=yo[:])

    nc.compile()
    return nc


# ---------------------------------------------------------------------------
# host side
# ---------------------------------------------------------------------------

def _block_w1_like(w):
    """[I, H] -> [I/128, 128, H] blocked so slab[i][p, k*128+c] =
    w[i*128+c, k*128+p] (i.e. w.T in lhsT-tile layout)."""
    i_sz, h = w.shape
    it = i_sz // P
    v = w.reshape(it, P, h // P, P)        # [i, c, k, p]
    return np.ascontiguousarray(v.transpose(0, 3, 2, 1)).reshape(it, P, h)


def _block_w2_doublerow(w2e):
    """[H, I] fp32 -> [H/128, 128, I/256, 2, 128] e4m3 with
    [hh, p, j, r, m] = w2[hh*128+m, j*256+r*128+p] * W2_FP8_SCALE."""
    hsz, i_sz = w2e.shape
    ht = hsz // P
    v = (w2e * W2_FP8_SCALE).reshape(ht, P, i_sz // 256, 2, P)
    # [hh, m, j, r, p] -> [hh, p, j, r, m]
    v = np.clip(v, -240.0, 240.0).transpose(0, 4, 2, 3, 1)
    return np.ascontiguousarray(np.asarray(v, dtype=ml_dtypes.float8_e4m3))


def _route(hs, gate):
    """Top-2 routing identical to the reference (jax on CPU)."""
    try:
        import jax
        import jax.numpy as jnp
        cpu = jax.devices("cpu")[0]
        with jax.default_device(cpu):
            logits = jnp.einsum('th,eh->te', jnp.asarray(hs), jnp.asarray(gate))
            probs = jax.nn.softmax(logits, axis=-1)
            topv, topi = jax.lax.top_k(probs, TOP_K)
            topv = topv / jnp.sum(topv, axis=-1, keepdims=True)
            return np.asarray(topi), np.asarray(topv, dtype=np.float32)
    except Exception:
        logits = hs.astype(np.float32) @ gate.astype(np.float32).T
        m = logits.max(axis=-1, keepdims=True)
        p = np.exp(logits - m)
        probs = p / p.sum(axis=-1, keepdims=True)
        topi = np.argsort(-probs, axis=-1, kind="stable")[:, :TOP_K]
        topv = np.take_along_axis(probs, topi, axis=-1)
        topv = topv / topv.sum(axis=-1, keepdims=True)
        return topi.astype(np.int64), topv.astype(np.float32)


# capacity layout: 4 chunks of 512; last chunk runs GEMM2 in fp8 DoubleRow
C_CAP = 2048
N_HI = 1536
GROUPS = ((512, 512), (512, 512))
LO_CHUNKS = frozenset({(1, 1)})

_PROG_CACHE = {}


def _get_program():
    key = (GROUPS, LO_CHUNKS)
    if key not in _PROG_CACHE:
        _PROG_CACHE[key] = build_program(GROUPS, LO_CHUNKS)
    return _PROG_CACHE[key]


def kernel(index, hidden_states, gate_w, w1, w3, w2, _trace=False):
    from concourse.bass_utils import run_bass_kernel_spmd

    idx = int(np.asarray(index))
    hs = np.asarray(hidden_states, dtype=np.float32)      # [T, H]
    t_num, h = hs.shape

    topi, topv = _route(hs, np.asarray(gate_w[idx], dtype=np.float32))
    flat_e = topi.ravel()                                  # [2T] pair expert
    flat_t = np.repeat(np.arange(t_num), TOP_K)            # [2T] pair token
    flat_w = topv.ravel().astype(np.float32)               # [2T] pair weight

    nc = _get_program()

    hs_bf = np.asarray(hs, dtype=ml_dtypes.bfloat16)
    in_maps = []
    # slot assignment: per expert, pairs in DESCENDING weight order; first
    # N_HI slots -> bf16 GEMM2, next C_CAP-N_HI -> fp8 GEMM2, rest -> host.
    pos = np.full(flat_e.size, -1, dtype=np.int64)
    for e in range(EXPERTS):
        sel = np.where(flat_e == e)[0]
        sel = sel[np.argsort(-flat_w[sel], kind="stable")]
        ndev = min(sel.size, C_CAP)
        pos[sel[:ndev]] = e * C_CAP + np.arange(ndev)

        tok = flat_t[sel[:ndev]]
        xpad = np.zeros((C_CAP, h), dtype=ml_dtypes.bfloat16)
        xpad[:ndev] = hs_bf[tok]
        wr = np.zeros((1, C_CAP), dtype=np.float32)
        wr[0, :ndev] = flat_w[sel[:ndev]]
        wr[0, N_HI:] /= W2_FP8_SCALE      # fold fp8 w2 unscale into weights
        w1b = _block_w1_like(np.asarray(w1[idx, e], dtype=ml_dtypes.bfloat16))
        w3b = _block_w1_like(np.asarray(w3[idx, e], dtype=ml_dtypes.bfloat16))
        w2_f32 = np.asarray(w2[idx, e], dtype=np.float32)
        in_maps.append({
            "xg": np.ascontiguousarray(xpad.T),
            "wb": np.ascontiguousarray(
                np.broadcast_to(wr, (P, C_CAP)).astype(np.float32)),
            "w13t": np.ascontiguousarray(
                np.concatenate([w1b, w3b], axis=2)),
            "w2t": _block_w1_like(np.asarray(w2[idx, e],
                                             dtype=ml_dtypes.bfloat16)),
            "w2t8": _block_w2_doublerow(w2_f32),
        })

    res = run_bass_kernel_spmd(nc, in_maps, core_ids=list(range(N_CORES)),
                               trace=False)
    # y_all[e*C_CAP + s] = output row (length H) of the pair in slot s
    y_all = np.concatenate(
        [np.asarray(r["out"], dtype=np.float32).T for r in res.results], axis=0)
    ovf = pos < 0                                          # capacity spill
    contrib = np.empty((pos.size, h), dtype=np.float32)
    contrib[~ovf] = y_all[pos[~ovf]]
    if ovf.any():
        for e in np.unique(flat_e[ovf]):
            m = ovf & (flat_e == e)
            x_e = hs[flat_t[m]]                            # [n, H] fp32
            h1 = x_e @ np.asarray(w1[idx, e], dtype=np.float32).T
            h3 = x_e @ np.asarray(w3[idx, e], dtype=np.float32).T
            hsw = (h1 / (1.0 + np.exp(-h1))) * h3
            contrib[m] = (hsw @ np.asarray(w2[idx, e], dtype=np.float32).T
                          ) * flat_w[m][:, None]
    out = contrib[0::2] + contrib[1::2]
    kernel._last_in_maps = in_maps
    return out


# revision 10
# speedup vs baseline: 1.0450x; 1.0450x over previous
"""Trainium2 Bass kernel for Mixtral-style MoE (8 experts, top-2, SwiGLU).

Strategy: expert-parallel across the 8 NeuronCores with host-side dispatch.
The router is tiny (8192x2048x8 = 0.27 GFLOP) and runs on host CPU with the
exact same jax ops as the reference (bitwise-matching top-2 selection).  Each
core owns one expert: the host gathers that expert's routed tokens (avg 2048
of the 16384 (token, expert) pairs), pads to a uniform capacity C=2048 so all
cores run the same program (SPMD), and the device does only the expert FFN:

  h1 = W1 @ x ; h3 = W3 @ x ; h = silu(h1) * h3 ; y = (W2 @ h) * pair_weight

in bf16 with fp32 PSUM accumulation (103 GFLOP/core vs 412 dense).  The host
scatter-adds each token's two expert partials.

Mixed precision: slots are filled per expert in DESCENDING pair-weight order.
The last 512 slots (the expert's lowest-weight pairs, mostly the rank-2
expert of well-routed tokens) run GEMM2 in fp8-e4m3 DoubleRow mode (2x PE
throughput): h is cast to e4m3 unscaled (|h| << 240), w2 is pre-scaled by
2048 on host (absmax*2048 = 222 < 240) and the 1/2048 unscale is folded into
the host-prepared pair-weight matrix.  Measured end-to-end rel err 1.2e-2
against the fp32 reference (gate 2e-2); bf16-only is 4.1e-3.

Token chunks are processed in PAIRS per weight pass: each streamed weight
tile issues two back-to-back matmuls (chunk A, chunk B) so the stationary-
operand load amortizes over 1024 moving columns, and weight HBM traffic
halves.  Other schedule details:
  - w1/w3 slabs are packed into ONE [it, 128, 2H] stream tile: one DMA + one
    PE semaphore wait per i-tile.
  - the stream is prefetched 2 tiles deep BEFORE the x-tile DMA block.
  - GEMM2's PSUM pool is double-buffered (no stall at hh boundaries).
  - the pair-weight broadcast [128, C] is precomputed on host and DMA'd
    outside the warmup critical path.

Layouts (host-prepared, per core e):
  xg    : [H, C]   bf16 gathered tokens, slot-ordered (weight-desc)
  wb    : [128, C] fp32 pair weight broadcast along partitions (0 on padding;
          divided by 2048 on fp8 slots)
  w13t  : [I/128, 128, 2H] bf16; first H cols w1 lhsT slab, last H w3
          (slab i row p, col k*128+c holds w[i*128+c, k*128+p])
  w2t   : [H/128, 128, I] bf16, same blocking for w2.T
  w2t8  : [H/128, 128, I/256, 2, 128] e4m3, DoubleRow blocking of w2.T*2048:
          [hh, p, j, r, m] = w2[hh*128+m, j*256+r*128+p]*2048
  out   : [H, C] fp32 partial outputs (host transposes/scatter-adds)
"""

import numpy as np
import ml_dtypes

import concourse.bass as bass
import concourse.mybir as mybir
import concourse.tile as tile
from concourse import bacc

P = 128
FP32 = mybir.dt.float32
BF16 = mybir.dt.bfloat16
E4 = mybir.dt.float8e4

# Full-problem constants
N_CORES = 8
NUM_TOKENS = 8192
HIDDEN = 2048
INTER = 4096
EXPERTS = 8
TOP_K = 2

W2_FP8_SCALE = 2048.0   # power of two; w2 absmax * 2048 = 222 < 240


def build_program(groups, lo_chunks=frozenset(), h=HIDDEN, i_sz=INTER):
    """groups: tuple of tuples of chunk sizes.  Each group is either
    (a,) / (a, b) with a,b <= 512, or (a, b, t) with t <= 256 (tail rider).
    lo_chunks: set of (group_idx, chunk_idx) whose GEMM2 runs in fp8
    DoubleRow (chunk size must be 512 and it must be even for those).
    """
    c_cap = sum(sum(g) for g in groups)
    kt = h // P
    it = i_sz // P
    ht = h // P

    nc = bacc.Bacc("TRN2", target_bir_lowering=False, debug=False)

    xg = nc.dram_tensor("xg", [h, c_cap], BF16, kind="ExternalInput").ap()
    wb_d = nc.dram_tensor("wb", [P, c_cap], FP32, kind="ExternalInput").ap()
    w13t = nc.dram_tensor("w13t", [it, P, 2 * h], BF16, kind="ExternalInput").ap()
    w2t = nc.dram_tensor("w2t", [ht, P, i_sz], BF16, kind="ExternalInput").ap()
    need_fp8 = bool(lo_chunks)
    if need_fp8:
        w2t8 = nc.dram_tensor("w2t8", [ht, P, it // 2, 2, P], E4,
                              kind="ExternalInput").ap()
    out_d = nc.dram_tensor("out", [h, c_cap], FP32, kind="ExternalOutput").ap()

    with tile.TileContext(nc) as tc:
        with (
            tc.tile_pool(name="const", bufs=1) as const_pool,
            tc.tile_pool(name="xpool", bufs=1) as x_pool,
            tc.tile_pool(name="hpool", bufs=1) as h_pool,
            tc.tile_pool(name="stream", bufs=3) as stream_pool,
            tc.tile_pool(name="w2stream", bufs=2) as w2_pool,
            tc.tile_pool(name="work", bufs=2) as work_pool,
            tc.tile_pool(name="opool", bufs=3) as o_pool,
            tc.tile_pool(name="psum1", bufs=1, space="PSUM") as psum1,
            tc.tile_pool(name="psum2", bufs=2, space="PSUM") as psum2,
        ):
            first_grp = True
            wb_full = None

            off = 0
            for gi, grp in enumerate(groups):
                cks = []
                for ck in grp:
                    cks.append((off, ck))
                    off += ck
                nch = len(cks)
                has_tail = nch == 3
                if has_tail:
                    assert cks[2][1] <= 256
                is_lo = [(gi, c) in lo_chunks for c in range(nch)]
                for c in range(nch):
                    if is_lo[c]:
                        assert cks[c][1] == 512 and it % 2 == 0

                # 2-deep prefetch of the merged w1/w3 stream, issued before
                # the x block so the first matmul's stationary tile is in
                # flight immediately.
                w13_pend = []

                def issue_w13(i):
                    t = stream_pool.tile([P, 2 * h], BF16, tag="w13s",
                                         name="w13s")
                    nc.sync.dma_start(out=t[:], in_=w13t[i])
                    w13_pend.append(t)

                issue_w13(0)
                issue_w13(1)

                # x tiles, k-major to match matmul consumption order
                xtb = [[None] * kt for _ in range(nch)]
                for k in range(kt):
                    for c, (o, ck) in enumerate(cks):
                        x = x_pool.tile([P, ck], BF16, tag=f"xtb{c}_{k}",
                                        name=f"xtb{c}_{k}")
                        nc.scalar.dma_start(
                            out=x[:], in_=xg[k * P:(k + 1) * P, o:o + ck])
                        xtb[c][k] = x

                # GEMM1 + SwiGLU
                h_sb = [[] for _ in range(nch)]     # bf16 h tiles (hi chunks)
                h8_sb = [[] for _ in range(nch)]    # e4m3 DoubleRow tiles
                for i in range(it):
                    if i + 2 < it:
                        issue_w13(i + 2)
                    w13s = w13_pend.pop(0)
                    h1_ps, h3_ps = [], []
                    for c, (_, ck) in enumerate(cks):
                        if c == 2:  # tail rider: one bank, two halves
                            hT = psum1.tile([P, 2 * ck], FP32, tag="hT",
                                            name="hT")
                            h1_ps.append(hT[:, 0:ck])
                            h3_ps.append(hT[:, ck:2 * ck])
                        else:
                            t1 = psum1.tile([P, ck], FP32, tag=f"h1_{c}",
                                            name=f"h1_{c}")
                            t3 = psum1.tile([P, ck], FP32, tag=f"h3_{c}",
                                            name=f"h3_{c}")
                            h1_ps.append(t1[:])
                            h3_ps.append(t3[:])
                    for k in range(kt):
                        for c in range(nch):
                            nc.tensor.matmul(out=h1_ps[c],
                                             lhsT=w13s[:, k * P:(k + 1) * P],
                                             rhs=xtb[c][k][:],
                                             start=(k == 0), stop=(k == kt - 1))
                    for k in range(kt):
                        for c in range(nch):
                            nc.tensor.matmul(
                                out=h3_ps[c],
                                lhsT=w13s[:, h + k * P:h + (k + 1) * P],
                                rhs=xtb[c][k][:],
                                start=(k == 0), stop=(k == kt - 1))
                    for c, (_, ck) in enumerate(cks):
                        sg = work_pool.tile([P, ck], FP32, tag=f"sg{c}",
                                            name=f"sg{c}")
                        nc.scalar.activation(
                            out=sg[:], in_=h1_ps[c],
                            func=mybir.ActivationFunctionType.Sigmoid)
                        sil = work_pool.tile([P, ck], FP32, tag=f"sil{c}",
                                             name=f"sil{c}")
                        nc.vector.tensor_tensor(out=sil[:], in0=sg[:],
                                                in1=h1_ps[c],
                                                op=mybir.AluOpType.mult)
                        if is_lo[c]:
                            # DoubleRow rhs tile: pair j holds i=2j (r=0) and
                            # i=2j+1 (r=1); h cast straight to e4m3 (no scale)
                            j, r = divmod(i, 2)
                            if r == 0:
                                h8 = h_pool.tile([P, 2, ck], E4,
                                                 tag=f"h8_{c}_{j}",
                                                 name=f"h8_{c}_{j}")
                                h8_sb[c].append(h8)
                            h8 = h8_sb[c][j]
                            nc.vector.tensor_tensor(out=h8[:, r, :], in0=sil[:],
                                                    in1=h3_ps[c],
                                                    op=mybir.AluOpType.mult)
                        else:
                            hcur = h_pool.tile([P, ck], BF16, tag=f"h{c}_{i}",
                                               name=f"h{c}_{i}")
                            nc.vector.tensor_tensor(out=hcur[:], in0=sil[:],
                                                    in1=h3_ps[c],
                                                    op=mybir.AluOpType.mult)
                            h_sb[c].append(hcur)

                if first_grp:
                    # pair-weight broadcast, precomputed on host; DMA'd here
                    # (first needed at the first yo multiply) so it stays off
                    # the scalar DMA ring's critical lead path.
                    wb_full = const_pool.tile([P, c_cap], FP32, tag="wb_full")
                    nc.scalar.dma_start(out=wb_full[:], in_=wb_d[:, :])
                    first_grp = False

                # GEMM2, w2 streams prefetched 2 deep
                any_hi = any(not lo for lo in is_lo)
                any_lo = any(is_lo)
                w2_pend = []
                w28_pend = []

                def issue_w2(hh):
                    if any_hi:
                        t = w2_pool.tile([P, i_sz], BF16, tag="w2s",
                                         name="w2s")
                        nc.sync.dma_start(out=t[:], in_=w2t[hh])
                        w2_pend.append(t)
                    if any_lo:
                        t8 = w2_pool.tile([P, it // 2, 2, P], E4, tag="w28s",
                                          name="w28s")
                        nc.sync.dma_start(out=t8[:], in_=w2t8[hh])
                        w28_pend.append(t8)

                issue_w2(0)
                issue_w2(1)
                for hh in range(ht):
                    if hh + 2 < ht:
                        issue_w2(hh + 2)
                    w2s = w2_pend.pop(0) if any_hi else None
                    w28s = w28_pend.pop(0) if any_lo else None
                    f_ps = [psum2.tile([P, ck], FP32, tag=f"f_{c}",
                                       name=f"f_{c}")
                            for c, (_, ck) in enumerate(cks)]
                    for c in range(nch):
                        if is_lo[c]:
                            continue
                        for i in range(it):
                            nc.tensor.matmul(out=f_ps[c][:],
                                             lhsT=w2s[:, i * P:(i + 1) * P],
                                             rhs=h_sb[c][i][:],
                                             start=(i == 0), stop=(i == it - 1))
                    for c in range(nch):
                        if not is_lo[c]:
                            continue
                        for j in range(it // 2):
                            nc.tensor.matmul(
                                out=f_ps[c][:], lhsT=w28s[:, j],
                                rhs=h8_sb[c][j][:],
                                start=(j == 0), stop=(j == it // 2 - 1),
                                perf_mode=mybir.MatmulPerfMode.DoubleRow)
                    for c, (o, ck) in enumerate(cks):
                        yo = o_pool.tile([P, ck], FP32, tag=f"yo{c}",
                                         name=f"yo{c}")
                        nc.vector.tensor_tensor(out=yo[:], in0=f_ps[c][:],
                                                in1=wb_full[:, o:o + ck],
                                                op=mybir.AluOpType.mult)
                        nc.scalar.dma_start(
                            out=out_d[hh * P:(hh + 1) * P, o:o + ck],
                            in_=yo[:])

    nc.compile()
    return nc


# ---------------------------------------------------------------------------
# host side
# ---------------------------------------------------------------------------

def _block_w1_like(w):
    """[I, H] -> [I/128, 128, H] blocked so slab[i][p, k*128+c] =
    w[i*128+c, k*128+p] (i.e. w.T in lhsT-tile layout)."""
    i_sz, h = w.shape
    it = i_sz // P
    v = w.reshape(it, P, h // P, P)        # [i, c, k, p]
    return np.ascontiguousarray(v.transpose(0, 3, 2, 1)).reshape(it, P, h)


def _block_w2_doublerow(w2e):
    """[H, I] fp32 -> [H/128, 128, I/256, 2, 128] e4m3 with
    [hh, p, j, r, m] = w2[hh*128+m, j*256+r*128+p] * W2_FP8_SCALE."""
    hsz, i_sz = w2e.shape
    ht = hsz // P
    v = (w2e * W2_FP8_SCALE).reshape(ht, P, i_sz // 256, 2, P)
    # [hh, m, j, r, p] -> [hh, p, j, r, m]
    v = np.clip(v, -240.0, 240.0).transpose(0, 4, 2, 3, 1)
    return np.ascontiguousarray(np.asarray(v, dtype=ml_dtypes.float8_e4m3))


def _route(hs, gate):
    """Top-2 routing identical to the reference (jax on CPU)."""
    try:
        import jax
        import jax.numpy as jnp
        cpu = jax.devices("cpu")[0]
        with jax.default_device(cpu):
            logits = jnp.einsum('th,eh->te', jnp.asarray(hs), jnp.asarray(gate))
            probs = jax.nn.softmax(logits, axis=-1)
            topv, topi = jax.lax.top_k(probs, TOP_K)
            topv = topv / jnp.sum(topv, axis=-1, keepdims=True)
            return np.asarray(topi), np.asarray(topv, dtype=np.float32)
    except Exception:
        logits = hs.astype(np.float32) @ gate.astype(np.float32).T
        m = logits.max(axis=-1, keepdims=True)
        p = np.exp(logits - m)
        probs = p / p.sum(axis=-1, keepdims=True)
        topi = np.argsort(-probs, axis=-1, kind="stable")[:, :TOP_K]
        topv = np.take_along_axis(probs, topi, axis=-1)
        topv = topv / topv.sum(axis=-1, keepdims=True)
        return topi.astype(np.int64), topv.astype(np.float32)


# capacity layout: 4 chunks of 512.  LO_CHUNKS would run GEMM2 in fp8
# DoubleRow (2x MAC rate, verified correct at rel err 1.2e-2) but measured
# NET SLOWER on hw: any fp8 DoubleRow work throttles the PE clock globally
# (~220ns -> 235ns per 512-col matmul across the whole kernel), costing more
# than the 2x gain on the 4% of work the 2e-2 error budget allows.  Disabled.
C_CAP = 2048
N_HI = 1536
GROUPS = ((512, 512), (512, 512))
LO_CHUNKS = frozenset()

_PROG_CACHE = {}


def _get_program():
    key = (GROUPS, LO_CHUNKS)
    if key not in _PROG_CACHE:
        _PROG_CACHE[key] = build_program(GROUPS, LO_CHUNKS)
    return _PROG_CACHE[key]


def kernel(index, hidden_states, gate_w, w1, w3, w2, _trace=False):
    from concourse.bass_utils import run_bass_kernel_spmd

    idx = int(np.asarray(index))
    hs = np.asarray(hidden_states, dtype=np.float32)      # [T, H]
    t_num, h = hs.shape

    topi, topv = _route(hs, np.asarray(gate_w[idx], dtype=np.float32))
    flat_e = topi.ravel()                                  # [2T] pair expert
    flat_t = np.repeat(np.arange(t_num), TOP_K)            # [2T] pair token
    flat_w = topv.ravel().astype(np.float32)               # [2T] pair weight

    nc = _get_program()

    hs_bf = np.asarray(hs, dtype=ml_dtypes.bfloat16)
    in_maps = []
    # slot assignment: per expert, pairs in DESCENDING weight order; first
    # N_HI slots -> bf16 GEMM2, next C_CAP-N_HI -> fp8 GEMM2, rest -> host.
    pos = np.full(flat_e.size, -1, dtype=np.int64)
    for e in range(EXPERTS):
        sel = np.where(flat_e == e)[0]
        sel = sel[np.argsort(-flat_w[sel], kind="stable")]
        ndev = min(sel.size, C_CAP)
        pos[sel[:ndev]] = e * C_CAP + np.arange(ndev)

        tok = flat_t[sel[:ndev]]
        xpad = np.zeros((C_CAP, h), dtype=ml_dtypes.bfloat16)
        xpad[:ndev] = hs_bf[tok]
        wr = np.zeros((1, C_CAP), dtype=np.float32)
        wr[0, :ndev] = flat_w[sel[:ndev]]
        if LO_CHUNKS:
            wr[0, N_HI:] /= W2_FP8_SCALE  # fold fp8 w2 unscale into weights
        w1b = _block_w1_like(np.asarray(w1[idx, e], dtype=ml_dtypes.bfloat16))
        w3b = _block_w1_like(np.asarray(w3[idx, e], dtype=ml_dtypes.bfloat16))
        im = {
            "xg": np.ascontiguousarray(xpad.T),
            "wb": np.ascontiguousarray(
                np.broadcast_to(wr, (P, C_CAP)).astype(np.float32)),
            "w13t": np.ascontiguousarray(
                np.concatenate([w1b, w3b], axis=2)),
            "w2t": _block_w1_like(np.asarray(w2[idx, e],
                                             dtype=ml_dtypes.bfloat16)),
        }
        if LO_CHUNKS:
            im["w2t8"] = _block_w2_doublerow(
                np.asarray(w2[idx, e], dtype=np.float32))
        in_maps.append(im)

    res = run_bass_kernel_spmd(nc, in_maps, core_ids=list(range(N_CORES)),
                               trace=False)
    # y_all[e*C_CAP + s] = output row (length H) of the pair in slot s
    y_all = np.concatenate(
        [np.asarray(r["out"], dtype=np.float32).T for r in res.results], axis=0)
    ovf = pos < 0                                          # capacity spill
    contrib = np.empty((pos.size, h), dtype=np.float32)
    contrib[~ovf] = y_all[pos[~ovf]]
    if ovf.any():
        for e in np.unique(flat_e[ovf]):
            m = ovf & (flat_e == e)
            x_e = hs[flat_t[m]]                            # [n, H] fp32
            h1 = x_e @ np.asarray(w1[idx, e], dtype=np.float32).T
            h3 = x_e @ np.asarray(w3[idx, e], dtype=np.float32).T
            hsw = (h1 / (1.0 + np.exp(-h1))) * h3
            contrib[m] = (hsw @ np.asarray(w2[idx, e], dtype=np.float32).T
                          ) * flat_w[m][:, None]
    out = contrib[0::2] + contrib[1::2]
    kernel._last_in_maps = in_maps
    return out
